# revision 55
# baseline (speedup 1.0000x reference)
"""BertWordPair pairwise-CE loss kernel for 8 Trainium2 NeuronCores.

Math (per (b,t) pair):
    proj = x @ W + b                      # only q_token / k_token columns used
    logits[m,n,c] = q_c[m] . k_c[n]
    nll[m,n] = logsumexp_c(logits) - logits[label]
    loss_bt  = sum(ww * nll) / sum(ww),   ww = class_weight[label] * mask
    out = sum_bt loss_bt

Device computes, per (b,t):
    num1  = sum_{m,n} ww * lse
    S_all = sum_{m,n} [mask] * logits'[label]         (label-selected sum)
fast path (cw = [a, w, ..., w]): the host pre-scales the class-0 q columns
by a/w, so logits'_0 = (a/w) logits_0 and num2 = w*S_all exactly; the
class-0 exp applies scale=w/a to recover the true softmax logit.
general path: per-class sums s_c,  num2 = sum_c cw[c]*s_c.
Host combines: den = sum ww ; loss = sum_bt (num1-num2)/den_bt.
Sharding: 32 (b,t) pairs split 4-per-core (data parallel, W replicated).

Schedule: stages are software-pipelined across (b,t) with per-engine
instruction streams interleaved at m-block granularity (logits chunks of
bt s-1 between projection db-blocks of bt s), logits flowing through
three 2-bank PSUM pools, and the Ln/num1 tail lagging one extra stage.
Exp/Ln/Identity are pinned to one activation table set
(natural_log_exp_and_others) so no per-bt table reloads occur. Deep tile
pools (exp x4, scr x8) keep the select/exp consumers from WAR-stalling
on pool rotation -- this was worth ~1.5x alone.
"""

import numpy as np
import ml_dtypes

B, T, L, H = 4, 8, 512, 768
INNER = 64
C = 6
NCORES = 8
BT_PER_CORE = (B * T) // NCORES  # 4
MB = L // 128                    # 4 m-blocks per (b,t)
HC = H // 128                    # 6 h-chunks

_BF16 = ml_dtypes.bfloat16
_F8 = ml_dtypes.float8_e4m3
# fp8 DoubleRow projection measured same-speed (DVE-bound, not PE-bound) at
# rel_err 1.1e-3 vs 3.6e-5 — keep the exact bf16 path
_FP8PROJ = False

_compiled = {}


def _patch_act_tables():
    """Bias the act-table-set chooser so Exp and Ln resolve to the one set
    that contains both (natural_log_exp_and_others); otherwise each Exp<->Ln
    transition reloads a table (~2.7us each, 8 per kernel body)."""
    import concourse.hw_specs as hw_specs
    import concourse.mybir as mybir

    for arch in ("gen3",):
        try:
            tables = hw_specs.get_activation_tables(arch)
        except Exception:
            continue
        for name, funcs in tables.items():
            if name != "natural_log_exp_and_others":
                funcs.discard(mybir.ActivationFunctionType.Exp)
                funcs.discard(mybir.ActivationFunctionType.Ln)
                funcs.discard(mybir.ActivationFunctionType.Identity)
                funcs.discard(mybir.ActivationFunctionType.Copy)


def _build_nc(fast_cw, repeat=1, ablate=(), b_zero=False, pool_tree=False,
              dev_lab=False, w_over_a=4.0, deep=True, fp8proj=False):
    import concourse.bacc as bacc
    import concourse.mybir as mybir
    from concourse.tile import TileContext

    _patch_act_tables()

    f32 = mybir.dt.float32
    bf16 = mybir.dt.bfloat16
    ADD = mybir.AluOpType.add
    MULT = mybir.AluOpType.mult
    EQ = mybir.AluOpType.is_equal

    nc = bacc.Bacc()
    _W_OVER_A = w_over_a

    # number of per-(b,t) scalars shipped out: num1 + S_all or 6 s_c
    nres = 2 if fast_cw else 7

    f8 = mybir.dt.float8e4
    if fp8proj:
        # projection in fp8e4m3 DoubleRow: h-chunks paired (3 pairs of 2x128),
        # W pre-scaled by 64 on host to clear the e4m3 denormal range
        wqk_d = nc.dram_tensor("wqk", [3, 2, 128, 768], f8, kind="ExternalInput")
        xt_d = nc.dram_tensor("xt", [BT_PER_CORE, 3, 2, 128, L], f8, kind="ExternalInput")
    else:
        wqk_d = nc.dram_tensor("wqk", [HC, 128, 768], bf16, kind="ExternalInput")
        xt_d = nc.dram_tensor("xt", [BT_PER_CORE, HC, 128, L], bf16, kind="ExternalInput")
    bias_d = nc.dram_tensor("bias", [128, 6], f32, kind="ExternalInput")
    if dev_lab:
        lab3_d = nc.dram_tensor("lab3", [BT_PER_CORE, MB, 128, L], bf16, kind="ExternalInput")
    else:
        lab3_d = nc.dram_tensor("lab3", [BT_PER_CORE, MB, 128, 2, L], bf16, kind="ExternalInput")
    ww_d = nc.dram_tensor("ww", [BT_PER_CORE, MB, 128, L], bf16, kind="ExternalInput")
    out_d = nc.dram_tensor("out", [1, nres * BT_PER_CORE], f32, kind="ExternalOutput")

    S = BT_PER_CORE * repeat

    with TileContext(nc) as tc:
        with (
            tc.tile_pool(name="const", bufs=1) as const_pool,
            tc.tile_pool(name="xt", bufs=3 if deep else 2) as xt_pool,
            tc.tile_pool(name="lab", bufs=3) as lab_pool,
            tc.tile_pool(name="lab0", bufs=2) as lab0_pool,
            tc.tile_pool(name="wwp", bufs=3) as ww_pool,
            tc.tile_pool(name="qk", bufs=4 if deep else 3) as qk_pool,
            tc.tile_pool(name="exp", bufs=4) as exp_pool,
            tc.tile_pool(name="se", bufs=3) as se_pool,
            tc.tile_pool(name="lse", bufs=3) as lse_pool,
            tc.tile_pool(name="tprod", bufs=3 if deep else 2) as t_pool,
            tc.tile_pool(name="scr", bufs=8) as scr_pool,
            tc.tile_pool(name="acc", bufs=3) as acc_pool,
            tc.tile_pool(name="res", bufs=1) as res_pool,
            tc.tile_pool(name="pproj", bufs=2, space="PSUM") as pproj_pool,
            tc.tile_pool(name="plogA", bufs=1, space="PSUM") as plogA_pool,
            tc.tile_pool(name="plogB", bufs=1, space="PSUM") as plogB_pool,
            tc.tile_pool(name="plogC", bufs=1, space="PSUM") as plogC_pool,
        ):
            if fp8proj:
                wqk_sb = const_pool.tile([128, 3, 2, 768], f8)
                nc.scalar.dma_start(out=wqk_sb, in_=wqk_d.rearrange("a b p d -> p a b d"))
            else:
                wqk_sb = const_pool.tile([128, HC, 768], bf16)
                nc.scalar.dma_start(out=wqk_sb, in_=wqk_d.rearrange("h p d -> p h d"))
            bias_sb = const_pool.tile([128, 6], f32)
            nc.sync.dma_start(out=bias_sb, in_=bias_d[:, :])
            ones_sb = const_pool.tile([128, 1], f32)
            nc.vector.memset(ones_sb, 1.0)
            res_sb = res_pool.tile([128, BT_PER_CORE, nres], f32)
            if ablate:
                nc.vector.memset(res_sb, 0.0)

            state = {}

            def front(s):
                """DMA the stage-s inputs (consumed by proj(s) and back(s))."""
                bt = s % BT_PER_CORE
                if fp8proj:
                    xt_sb = xt_pool.tile([128, 3, 2, L], f8, tag="xt")
                    nc.sync.dma_start(out=xt_sb, in_=xt_d[bt].rearrange("a b p l -> p a b l"))
                else:
                    xt_sb = xt_pool.tile([128, HC, L], bf16, tag="xt")
                    nc.sync.dma_start(out=xt_sb, in_=xt_d[bt].rearrange("h p l -> p h l"))
                if dev_lab:
                    lab_sb = lab0_pool.tile([128, MB, L], bf16, tag="lab0")
                    nc.scalar.dma_start(out=lab_sb, in_=lab3_d[bt].rearrange("m p l -> p m l"))
                else:
                    lab_sb = lab_pool.tile([128, MB, 2, L], bf16, tag="lab")
                    nc.scalar.dma_start(out=lab_sb, in_=lab3_d[bt].rearrange("m p j l -> p m j l"))
                ww_sb = ww_pool.tile([128, MB, L], bf16, tag="ww")
                nc.sync.dma_start(out=ww_sb, in_=ww_d[bt].rearrange("m p l -> p m l"))
                state[s] = [xt_sb, lab_sb, ww_sb, None]

            def proj_part(s, dbs):
                """Projection for stage s, db blocks `dbs` of the 768 columns."""
                xt_sb = state[s][0]
                if state[s][3] is None:
                    qk_sb = qk_pool.tile([128, 6, L], bf16, tag="qk")
                    state[s][3] = qk_sb
                qk_sb = state[s][3]
                unscale = 1.0 / 64.0 if fp8proj else 1.0
                for db in dbs:
                    pp = pproj_pool.tile([128, L], f32, tag="proj")
                    if fp8proj:
                        import concourse.mybir as _mybir
                        for pr in range(3):
                            nc.tensor.matmul(
                                pp,
                                lhsT=wqk_sb[:, pr, :, db * 128:(db + 1) * 128],
                                rhs=xt_sb[:, pr, :, :],
                                start=(pr == 0),
                                stop=(pr == 2),
                                perf_mode=_mybir.MatmulPerfMode.DoubleRow,
                            )
                    else:
                        for hc in range(HC):
                            nc.tensor.matmul(
                                pp,
                                lhsT=wqk_sb[:, hc, db * 128:(db + 1) * 128],
                                rhs=xt_sb[:, hc, :],
                                start=(hc == 0),
                                stop=(hc == HC - 1),
                            )
                    if b_zero:
                        nc.scalar.activation(
                            out=qk_sb[:, db, :], in_=pp,
                            func=mybir.ActivationFunctionType.Identity,
                            scale=unscale,
                        )
                    else:
                        nc.scalar.activation(
                            out=qk_sb[:, db, :], in_=pp,
                            func=mybir.ActivationFunctionType.Identity,
                            bias=bias_sb[:, db:db + 1], scale=unscale,
                        )

            bstate = {}

            def back_mb(s, mb):
                """Logits, exp, select, and sumexp tree for (stage s, block mb)."""
                _, lab_sb, ww_sb, qk_sb = state[s]
                if dev_lab and mb == 0:
                    # expand the shipped labels into the 2 shifted compare
                    # planes (lab - j) consumed by the 2-class select STTs
                    lab3_sb = lab_pool.tile([128, MB, 2, L], bf16, tag="lab")
                    nc.vector.tensor_copy(lab3_sb[:, :, 0, :], lab_sb)
                    nc.vector.tensor_scalar_add(
                        lab3_sb[:, :, 1, :], lab_sb, -1.0,
                    )
                    state[s][1] = lab3_sb
                    lab_sb = lab3_sb
                elif dev_lab:
                    lab_sb = state[s][1]
                if mb == 0:
                    if fast_cw:
                        accS = acc_pool.tile([128, 3 * MB], f32, tag="accS")
                        accs = (accS,)
                    else:
                        acc24 = acc_pool.tile([128, MB * 6], f32, tag="a24")
                        accs = (acc24,)
                    se_sb = se_pool.tile([128, MB, L], bf16, tag="se")
                    bstate[s] = (accs, se_sb)
                accs, se_sb = bstate[s]

                exp_sb = exp_pool.tile([128, 6, L], bf16, tag="exp")
                for t, pool in ((0, plogA_pool), (1, plogB_pool), (2, plogC_pool)):
                    pl = pool.tile([128, 2, L], f32, tag=f"log{t}")
                    for cc in (() if "logits" in ablate else range(2)):
                        c = 2 * t + cc
                        qpart = (c % 2) * 64
                        nc.tensor.matmul(
                            pl[:, cc, :],
                            lhsT=qk_sb[qpart:qpart + 64, c // 2, mb * 128:(mb + 1) * 128],
                            rhs=qk_sb[qpart:qpart + 64, 3 + c // 2, :],
                            start=True, stop=True,
                        )
                    # exp -> planar bf16 [c, n]. In the fast-cw path the host
                    # pre-scaled the class-0 q columns by cw0/w so the select
                    # accumulates cw[lab]/w * logit directly; the exp for
                    # class 0 undoes that with its free affine scale.
                    if "exp" in ablate:
                        pass
                    elif fast_cw and t == 0:
                        nc.scalar.activation(
                            out=exp_sb[:, 0:1, :], in_=pl[:, 0:1, :],
                            func=mybir.ActivationFunctionType.Exp,
                            scale=float(_W_OVER_A),
                        )
                        nc.scalar.activation(
                            out=exp_sb[:, 1:2, :], in_=pl[:, 1:2, :],
                            func=mybir.ActivationFunctionType.Exp,
                        )
                    else:
                        nc.scalar.activation(
                            out=exp_sb[:, 2 * t:2 * t + 2, :],
                            in_=pl,
                            func=mybir.ActivationFunctionType.Exp,
                        )
                    # label-selected logit sums: (lab-j == 2t) <=> lab == 2t+j
                    if "select" in ablate:
                        pass
                    elif fast_cw:
                        scr = scr_pool.tile([128, 2, L], bf16, tag="scr")
                        nc.vector.scalar_tensor_tensor(
                            out=scr,
                            in0=lab_sb[:, mb, :, :],
                            scalar=float(2 * t),
                            in1=pl,
                            op0=EQ, op1=MULT,
                            accum_out=accs[0][:, 3 * mb + t:3 * mb + t + 1],
                        )
                    else:
                        for cc in range(2):
                            c = 2 * t + cc
                            scr1 = scr_pool.tile([128, L], bf16, tag="scr1")
                            nc.vector.scalar_tensor_tensor(
                                out=scr1,
                                in0=lab_sb[:, mb, 0, :],
                                scalar=float(c),
                                in1=pl[:, cc, :],
                                op0=EQ, op1=MULT,
                                accum_out=accs[0][:, mb * 6 + c:mb * 6 + c + 1],
                            )
                # sumexp: 2x-mode TT add level 1 on DVE, levels 2-3 on GpSimd
                if "tree" in ablate:
                    return
                with nc.allow_low_precision("bf16 sumexp"):
                    s3 = scr_pool.tile([128, 3, L], bf16, tag="s3")
                    nc.vector.tensor_tensor(s3, exp_sb[:, 0:3, :], exp_sb[:, 3:6, :], op=ADD)
                    sa = scr_pool.tile([128, L], bf16, tag="sa")
                    eng2 = nc.gpsimd if pool_tree else nc.vector
                    eng2.tensor_tensor(sa, s3[:, 0, :], s3[:, 1, :], op=ADD)
                    eng2.tensor_tensor(se_sb[:, mb, :], sa, s3[:, 2, :], op=ADD)

            def tail_ln(s):
                """Ln over the accumulated sumexp of stage s (ScalarE only)."""
                if "tail" in ablate:
                    return
                _, se_sb = bstate[s]
                lse_sb = lse_pool.tile([128, MB, L], bf16, tag="lse")
                nc.scalar.activation(
                    out=lse_sb.rearrange("p m l -> p (m l)"),
                    in_=se_sb.rearrange("p m l -> p (m l)"),
                    func=mybir.ActivationFunctionType.Ln,
                )
                bstate[s] = (bstate[s][0], lse_sb)

            def tail_dve(s):
                """num1 accumulation + select reduces for stage s (VectorE)."""
                bt = s % BT_PER_CORE
                _, lab_sb, ww_sb, _ = state.pop(s)
                if "tail" in ablate:
                    bstate.pop(s, None)
                    return
                accs, lse_sb = bstate.pop(s)
                # num1 = sum(ww * lse): single fused STT with accumulate
                t_sb = t_pool.tile([128, MB * L], bf16, tag="t")
                nc.vector.scalar_tensor_tensor(
                    out=t_sb,
                    in0=ww_sb.rearrange("p m l -> p (m l)"),
                    scalar=1.0,
                    in1=lse_sb.rearrange("p m l -> p (m l)"),
                    op0=MULT, op1=MULT,
                    accum_out=res_sb[:, bt, 0:1],
                )
                if "select" in ablate:
                    pass
                elif fast_cw:
                    nc.vector.tensor_reduce(
                        out=res_sb[:, bt, 1:2], in_=accs[0],
                        axis=mybir.AxisListType.X, op=ADD,
                    )
                else:
                    nc.vector.tensor_reduce(
                        out=res_sb[:, bt, 1:7],
                        in_=accs[0].rearrange("p (m c) -> p c m", c=6),
                        axis=mybir.AxisListType.X, op=ADD,
                    )

            # Software pipeline, 2 stages deep. Per-engine instruction order
            # interleaves stage s-1 consumption with stage s projection at mb
            # granularity so no engine's in-order stream blocks another:
            #   PE : [logits(s-1,mb)] [proj(s,dbs)] x4
            #   ACT: [exp(s-1)] [qk copies(s)] x4, then Ln(s-1)
            #   DVE: [num1/reduce(s-2)] [selects(s-1), tree L1] x4
            #   Pool: tree L2/L3(s-1)
            DBS = ((0, 1), (2, 3), (4,), (5,))
            for s in range(S + 1):
                if s < S:
                    front(s)
                for mb in range(MB):
                    if s >= 1:
                        back_mb(s - 1, mb)
                        if mb == 0 and s >= 2:
                            tail_dve(s - 2)
                    if s < S:
                        proj_part(s, DBS[mb])
                if s >= 1:
                    tail_ln(s - 1)
            tail_dve(S - 1)

            # cross-partition reduce via ones-matmul (fp32, tiny)
            pout = pproj_pool.tile([1, nres * BT_PER_CORE], f32, tag="proj")
            nc.tensor.matmul(
                pout,
                lhsT=ones_sb[:, :],
                rhs=res_sb.rearrange("p b k -> p (b k)"),
                start=True, stop=True,
            )
            out_sb = res_pool.tile([1, nres * BT_PER_CORE], f32)
            nc.vector.tensor_copy(out_sb, pout)
            nc.sync.dma_start(out=out_d[:, :], in_=out_sb)

    nc.compile()
    nc.finalize()
    return nc


def _prep_core_inputs(x, W, b, class_weight, labels, mask):
    """Host-side prep. Returns (in_maps list of 8 dicts, den[32] float64)."""
    x32 = np.ascontiguousarray(np.asarray(x, np.float32).reshape(B * T, L, H))
    labels32 = np.asarray(labels).reshape(B * T, L, L)
    mask32 = np.asarray(mask).reshape(B * T, L, L)

    cw = np.asarray(class_weight, np.float32)
    fast_cw = bool(np.all(cw[1:] == cw[1]))

    Wr = np.asarray(W, np.float32).reshape(H, C, 4, INNER)
    Wq = Wr[:, :, 0, :].reshape(H, C, INNER).copy()
    br = np.asarray(b, np.float32).reshape(C, 4, INNER)
    bq = br[:, 0, :].copy()
    if fast_cw:
        # pre-scale class-0 q so the select accumulates cw[lab]/w * logit;
        # the device exp undoes this with scale=w/a (exact for a/w = 0.25)
        Wq[:, 0, :] *= cw[0] / cw[1]
        bq[0, :] *= cw[0] / cw[1]
    Wq = Wq.reshape(H, C * INNER)
    Wk = Wr[:, :, 2, :].reshape(H, C * INNER)
    if _FP8PROJ:
        wqk = np.ascontiguousarray(
            (np.concatenate([Wq, Wk], axis=1) * 64.0).reshape(3, 2, 128, 768)
        ).astype(_F8)
    else:
        wqk = np.ascontiguousarray(
            np.concatenate([Wq, Wk], axis=1).reshape(HC, 128, 768)
        ).astype(_BF16)

    br = np.concatenate([bq.ravel(), br[:, 2, :].ravel()])
    bias = np.ascontiguousarray(br.reshape(6, 128).T).astype(np.float32)
    ww_all = (cw[labels32] * mask32).astype(np.float32)          # [32, L, L]
    den = ww_all.astype(np.float64).reshape(B * T, -1).sum(axis=1)

    # masked labels: mask==0 positions get +32 so they never match any class;
    # lab3[..., j, :] = lab' - j for the 2-plane chunked select
    labp = (labels32 + 32 * (1 - mask32)).astype(np.float32)
    j2 = np.arange(2, dtype=np.float32).reshape(1, 1, 2, 1)

    in_maps = []
    for core in range(NCORES):
        sl = slice(core * BT_PER_CORE, (core + 1) * BT_PER_CORE)
        if _FP8PROJ:
            xt = np.ascontiguousarray(
                x32[sl].transpose(0, 2, 1).reshape(BT_PER_CORE, 3, 2, 128, L)
            ).astype(_F8)
        else:
            xt = np.ascontiguousarray(
                x32[sl].transpose(0, 2, 1).reshape(BT_PER_CORE, HC, 128, L)
            ).astype(_BF16)
        lab3 = np.ascontiguousarray(
            labp[sl].reshape(BT_PER_CORE, MB, 128, 1, L) - j2[None]
        ).astype(_BF16)
        ww_s = np.ascontiguousarray(
            ww_all[sl].reshape(BT_PER_CORE, MB, 128, L)
        ).astype(_BF16)
        in_maps.append({"wqk": wqk, "bias": bias, "xt": xt, "lab3": lab3, "ww": ww_s})
    return in_maps, den


def kernel(x, W, b, class_weight, labels, mask):
    from concourse.bass_utils import run_bass_kernel_spmd

    cw = np.asarray(class_weight, np.float64)
    fast_cw = bool(np.all(cw[1:] == cw[1]))
    b_zero = bool(np.all(np.asarray(b) == 0.0))
    w_over_a = float(cw[1] / cw[0]) if fast_cw else 4.0
    key = ("nc", fast_cw, b_zero, w_over_a)
    if key not in _compiled:
        _compiled[key] = _build_nc(fast_cw, b_zero=b_zero, w_over_a=w_over_a)
    nc = _compiled[key]

    in_maps, den = _prep_core_inputs(x, W, b, class_weight, labels, mask)
    res = run_bass_kernel_spmd(nc, in_maps, core_ids=list(range(NCORES)))
    _compiled["last_res"] = res

    nres = 2 if fast_cw else 7
    loss = 0.0
    for core in range(NCORES):
        out = np.asarray(res.results[core]["out"], np.float64).reshape(BT_PER_CORE, nres)
        for i in range(BT_PER_CORE):
            num1 = out[i, 0]
            if fast_cw:
                num2 = cw[1] * out[i, 1]
            else:
                num2 = float(cw @ out[i, 1:7])
            d = max(den[core * BT_PER_CORE + i], 1e-9)
            loss += (num1 - num2) / d
    return np.float32(loss)


# revision 57
# speedup vs baseline: 1.0073x; 1.0073x over previous
"""BertWordPair pairwise-CE loss kernel for 8 Trainium2 NeuronCores.

Math (per (b,t) pair):
    proj = x @ W + b                      # only q_token / k_token columns used
    logits[m,n,c] = q_c[m] . k_c[n]
    nll[m,n] = logsumexp_c(logits) - logits[label]
    loss_bt  = sum(ww * nll) / sum(ww),   ww = class_weight[label] * mask
    out = sum_bt loss_bt

Device computes, per (b,t):
    num1  = sum_{m,n} ww * lse
    S_all = sum_{m,n} [mask] * logits'[label]         (label-selected sum)
fast path (cw = [a, w, ..., w]): the host pre-scales the class-0 q columns
by a/w, so logits'_0 = (a/w) logits_0 and num2 = w*S_all exactly; the
class-0 exp applies scale=w/a to recover the true softmax logit.
general path: per-class sums s_c,  num2 = sum_c cw[c]*s_c.
Host combines: den = sum ww ; loss = sum_bt (num1-num2)/den_bt.
Sharding: 32 (b,t) pairs split 4-per-core (data parallel, W replicated).

Schedule: stages are software-pipelined across (b,t) with per-engine
instruction streams interleaved at m-block granularity (logits chunks of
bt s-1 between projection db-blocks of bt s), logits flowing through
three 2-bank PSUM pools, and the Ln/num1 tail lagging one extra stage.
Exp/Ln/Identity are pinned to one activation table set
(natural_log_exp_and_others) so no per-bt table reloads occur. Deep tile
pools (exp x4, scr x8) keep the select/exp consumers from WAR-stalling
on pool rotation -- this was worth ~1.5x alone.
"""

import numpy as np
import ml_dtypes

B, T, L, H = 4, 8, 512, 768
INNER = 64
C = 6
NCORES = 8
BT_PER_CORE = (B * T) // NCORES  # 4
MB = L // 128                    # 4 m-blocks per (b,t)
HC = H // 128                    # 6 h-chunks

_BF16 = ml_dtypes.bfloat16
_F8 = ml_dtypes.float8_e4m3
# fp8 DoubleRow projection measured same-speed (DVE-bound, not PE-bound) at
# rel_err 1.1e-3 vs 3.6e-5 — keep the exact bf16 path
_FP8PROJ = False

_compiled = {}


def _patch_act_tables():
    """Bias the act-table-set chooser so Exp and Ln resolve to the one set
    that contains both (natural_log_exp_and_others); otherwise each Exp<->Ln
    transition reloads a table (~2.7us each, 8 per kernel body)."""
    import concourse.hw_specs as hw_specs
    import concourse.mybir as mybir

    for arch in ("gen3",):
        try:
            tables = hw_specs.get_activation_tables(arch)
        except Exception:
            continue
        for name, funcs in tables.items():
            if name != "natural_log_exp_and_others":
                funcs.discard(mybir.ActivationFunctionType.Exp)
                funcs.discard(mybir.ActivationFunctionType.Ln)
                funcs.discard(mybir.ActivationFunctionType.Identity)
                funcs.discard(mybir.ActivationFunctionType.Copy)


def _build_nc(fast_cw, repeat=1, ablate=(), b_zero=False, pool_tree=False,
              dev_lab=False, w_over_a=4.0, deep=True, fp8proj=False,
              qk_on_dve=False):
    import concourse.bacc as bacc
    import concourse.mybir as mybir
    from concourse.tile import TileContext

    _patch_act_tables()

    f32 = mybir.dt.float32
    bf16 = mybir.dt.bfloat16
    ADD = mybir.AluOpType.add
    MULT = mybir.AluOpType.mult
    EQ = mybir.AluOpType.is_equal

    nc = bacc.Bacc()
    _W_OVER_A = w_over_a

    # number of per-(b,t) scalars shipped out: num1 + S_all or 6 s_c
    nres = 2 if fast_cw else 7

    f8 = mybir.dt.float8e4
    if fp8proj:
        # projection in fp8e4m3 DoubleRow: h-chunks paired (3 pairs of 2x128),
        # W pre-scaled by 64 on host to clear the e4m3 denormal range
        wqk_d = nc.dram_tensor("wqk", [3, 2, 128, 768], f8, kind="ExternalInput")
        xt_d = nc.dram_tensor("xt", [BT_PER_CORE, 3, 2, 128, L], f8, kind="ExternalInput")
    else:
        wqk_d = nc.dram_tensor("wqk", [HC, 128, 768], bf16, kind="ExternalInput")
        xt_d = nc.dram_tensor("xt", [BT_PER_CORE, HC, 128, L], bf16, kind="ExternalInput")
    bias_d = nc.dram_tensor("bias", [128, 6], f32, kind="ExternalInput")
    if dev_lab:
        lab3_d = nc.dram_tensor("lab3", [BT_PER_CORE, MB, 128, L], bf16, kind="ExternalInput")
    else:
        lab3_d = nc.dram_tensor("lab3", [BT_PER_CORE, MB, 128, 2, L], bf16, kind="ExternalInput")
    ww_d = nc.dram_tensor("ww", [BT_PER_CORE, MB, 128, L], bf16, kind="ExternalInput")
    out_d = nc.dram_tensor("out", [1, nres * BT_PER_CORE], f32, kind="ExternalOutput")

    S = BT_PER_CORE * repeat

    with TileContext(nc) as tc:
        with (
            tc.tile_pool(name="const", bufs=1) as const_pool,
            tc.tile_pool(name="xt", bufs=3 if deep else 2) as xt_pool,
            tc.tile_pool(name="lab", bufs=3) as lab_pool,
            tc.tile_pool(name="lab0", bufs=2) as lab0_pool,
            tc.tile_pool(name="wwp", bufs=3) as ww_pool,
            tc.tile_pool(name="qk", bufs=4 if deep else 3) as qk_pool,
            tc.tile_pool(name="exp", bufs=4) as exp_pool,
            tc.tile_pool(name="se", bufs=3) as se_pool,
            tc.tile_pool(name="lse", bufs=3) as lse_pool,
            tc.tile_pool(name="tprod", bufs=3 if deep else 2) as t_pool,
            tc.tile_pool(name="scr", bufs=8) as scr_pool,
            tc.tile_pool(name="acc", bufs=3) as acc_pool,
            tc.tile_pool(name="res", bufs=1) as res_pool,
            tc.tile_pool(name="pproj", bufs=2, space="PSUM") as pproj_pool,
            tc.tile_pool(name="plogA", bufs=1, space="PSUM") as plogA_pool,
            tc.tile_pool(name="plogB", bufs=1, space="PSUM") as plogB_pool,
            tc.tile_pool(name="plogC", bufs=1, space="PSUM") as plogC_pool,
        ):
            if fp8proj:
                wqk_sb = const_pool.tile([128, 3, 2, 768], f8)
                nc.scalar.dma_start(out=wqk_sb, in_=wqk_d.rearrange("a b p d -> p a b d"))
            else:
                wqk_sb = const_pool.tile([128, HC, 768], bf16)
                nc.scalar.dma_start(out=wqk_sb, in_=wqk_d.rearrange("h p d -> p h d"))
            bias_sb = const_pool.tile([128, 6], f32)
            nc.sync.dma_start(out=bias_sb, in_=bias_d[:, :])
            ones_sb = const_pool.tile([128, 1], f32)
            nc.vector.memset(ones_sb, 1.0)
            res_sb = res_pool.tile([128, BT_PER_CORE, nres], f32)
            if ablate:
                nc.vector.memset(res_sb, 0.0)

            state = {}

            def front(s):
                """DMA the stage-s inputs (consumed by proj(s) and back(s))."""
                bt = s % BT_PER_CORE
                if fp8proj:
                    xt_sb = xt_pool.tile([128, 3, 2, L], f8, tag="xt")
                    nc.sync.dma_start(out=xt_sb, in_=xt_d[bt].rearrange("a b p l -> p a b l"))
                else:
                    xt_sb = xt_pool.tile([128, HC, L], bf16, tag="xt")
                    nc.sync.dma_start(out=xt_sb, in_=xt_d[bt].rearrange("h p l -> p h l"))
                if dev_lab:
                    lab_sb = lab0_pool.tile([128, MB, L], bf16, tag="lab0")
                    nc.scalar.dma_start(out=lab_sb, in_=lab3_d[bt].rearrange("m p l -> p m l"))
                else:
                    lab_sb = lab_pool.tile([128, MB, 2, L], bf16, tag="lab")
                    nc.scalar.dma_start(out=lab_sb, in_=lab3_d[bt].rearrange("m p j l -> p m j l"))
                ww_sb = ww_pool.tile([128, MB, L], bf16, tag="ww")
                nc.sync.dma_start(out=ww_sb, in_=ww_d[bt].rearrange("m p l -> p m l"))
                state[s] = [xt_sb, lab_sb, ww_sb, None]

            def proj_part(s, dbs):
                """Projection for stage s, db blocks `dbs` of the 768 columns."""
                xt_sb = state[s][0]
                if state[s][3] is None:
                    qk_sb = qk_pool.tile([128, 6, L], bf16, tag="qk")
                    state[s][3] = qk_sb
                qk_sb = state[s][3]
                unscale = 1.0 / 64.0 if fp8proj else 1.0
                for db in dbs:
                    pp = pproj_pool.tile([128, L], f32, tag="proj")
                    if fp8proj:
                        import concourse.mybir as _mybir
                        for pr in range(3):
                            nc.tensor.matmul(
                                pp,
                                lhsT=wqk_sb[:, pr, :, db * 128:(db + 1) * 128],
                                rhs=xt_sb[:, pr, :, :],
                                start=(pr == 0),
                                stop=(pr == 2),
                                perf_mode=_mybir.MatmulPerfMode.DoubleRow,
                            )
                    else:
                        for hc in range(HC):
                            nc.tensor.matmul(
                                pp,
                                lhsT=wqk_sb[:, hc, db * 128:(db + 1) * 128],
                                rhs=xt_sb[:, hc, :],
                                start=(hc == 0),
                                stop=(hc == HC - 1),
                            )
                    if b_zero and qk_on_dve:
                        nc.vector.tensor_copy(qk_sb[:, db, :], pp)
                    elif b_zero:
                        nc.scalar.activation(
                            out=qk_sb[:, db, :], in_=pp,
                            func=mybir.ActivationFunctionType.Identity,
                            scale=unscale,
                        )
                    else:
                        nc.scalar.activation(
                            out=qk_sb[:, db, :], in_=pp,
                            func=mybir.ActivationFunctionType.Identity,
                            bias=bias_sb[:, db:db + 1], scale=unscale,
                        )

            bstate = {}

            def back_mb(s, mb):
                """Logits, exp, select, and sumexp tree for (stage s, block mb)."""
                _, lab_sb, ww_sb, qk_sb = state[s]
                if dev_lab and mb == 0:
                    # expand the shipped labels into the 2 shifted compare
                    # planes (lab - j) consumed by the 2-class select STTs
                    lab3_sb = lab_pool.tile([128, MB, 2, L], bf16, tag="lab")
                    nc.vector.tensor_copy(lab3_sb[:, :, 0, :], lab_sb)
                    nc.vector.tensor_scalar_add(
                        lab3_sb[:, :, 1, :], lab_sb, -1.0,
                    )
                    state[s][1] = lab3_sb
                    lab_sb = lab3_sb
                elif dev_lab:
                    lab_sb = state[s][1]
                if mb == 0:
                    if fast_cw:
                        accS = acc_pool.tile([128, 3 * MB], f32, tag="accS")
                        accs = (accS,)
                    else:
                        acc24 = acc_pool.tile([128, MB * 6], f32, tag="a24")
                        accs = (acc24,)
                    se_sb = se_pool.tile([128, MB, L], bf16, tag="se")
                    bstate[s] = (accs, se_sb)
                accs, se_sb = bstate[s]

                exp_sb = exp_pool.tile([128, 6, L], bf16, tag="exp")
                for t, pool in ((0, plogA_pool), (1, plogB_pool), (2, plogC_pool)):
                    pl = pool.tile([128, 2, L], f32, tag=f"log{t}")
                    for cc in (() if "logits" in ablate else range(2)):
                        c = 2 * t + cc
                        qpart = (c % 2) * 64
                        nc.tensor.matmul(
                            pl[:, cc, :],
                            lhsT=qk_sb[qpart:qpart + 64, c // 2, mb * 128:(mb + 1) * 128],
                            rhs=qk_sb[qpart:qpart + 64, 3 + c // 2, :],
                            start=True, stop=True,
                        )
                    # exp -> planar bf16 [c, n]. In the fast-cw path the host
                    # pre-scaled the class-0 q columns by cw0/w so the select
                    # accumulates cw[lab]/w * logit directly; the exp for
                    # class 0 undoes that with its free affine scale.
                    if "exp" in ablate:
                        pass
                    elif fast_cw and t == 0:
                        nc.scalar.activation(
                            out=exp_sb[:, 0:1, :], in_=pl[:, 0:1, :],
                            func=mybir.ActivationFunctionType.Exp,
                            scale=float(_W_OVER_A),
                        )
                        nc.scalar.activation(
                            out=exp_sb[:, 1:2, :], in_=pl[:, 1:2, :],
                            func=mybir.ActivationFunctionType.Exp,
                        )
                    else:
                        nc.scalar.activation(
                            out=exp_sb[:, 2 * t:2 * t + 2, :],
                            in_=pl,
                            func=mybir.ActivationFunctionType.Exp,
                        )
                    # label-selected logit sums: (lab-j == 2t) <=> lab == 2t+j
                    if "select" in ablate:
                        pass
                    elif fast_cw:
                        scr = scr_pool.tile([128, 2, L], bf16, tag="scr")
                        nc.vector.scalar_tensor_tensor(
                            out=scr,
                            in0=lab_sb[:, mb, :, :],
                            scalar=float(2 * t),
                            in1=pl,
                            op0=EQ, op1=MULT,
                            accum_out=accs[0][:, 3 * mb + t:3 * mb + t + 1],
                        )
                    else:
                        for cc in range(2):
                            c = 2 * t + cc
                            scr1 = scr_pool.tile([128, L], bf16, tag="scr1")
                            nc.vector.scalar_tensor_tensor(
                                out=scr1,
                                in0=lab_sb[:, mb, 0, :],
                                scalar=float(c),
                                in1=pl[:, cc, :],
                                op0=EQ, op1=MULT,
                                accum_out=accs[0][:, mb * 6 + c:mb * 6 + c + 1],
                            )
                # sumexp: 2x-mode TT add level 1 on DVE, levels 2-3 on GpSimd
                if "tree" in ablate:
                    return
                with nc.allow_low_precision("bf16 sumexp"):
                    s3 = scr_pool.tile([128, 3, L], bf16, tag="s3")
                    nc.vector.tensor_tensor(s3, exp_sb[:, 0:3, :], exp_sb[:, 3:6, :], op=ADD)
                    sa = scr_pool.tile([128, L], bf16, tag="sa")
                    eng2 = nc.gpsimd if pool_tree else nc.vector
                    eng2.tensor_tensor(sa, s3[:, 0, :], s3[:, 1, :], op=ADD)
                    eng2.tensor_tensor(se_sb[:, mb, :], sa, s3[:, 2, :], op=ADD)

            def tail_ln(s):
                """Ln over the accumulated sumexp of stage s (ScalarE only)."""
                if "tail" in ablate:
                    return
                _, se_sb = bstate[s]
                lse_sb = lse_pool.tile([128, MB, L], bf16, tag="lse")
                nc.scalar.activation(
                    out=lse_sb.rearrange("p m l -> p (m l)"),
                    in_=se_sb.rearrange("p m l -> p (m l)"),
                    func=mybir.ActivationFunctionType.Ln,
                )
                bstate[s] = (bstate[s][0], lse_sb)

            def tail_dve(s):
                """num1 accumulation + select reduces for stage s (VectorE)."""
                bt = s % BT_PER_CORE
                _, lab_sb, ww_sb, _ = state.pop(s)
                if "tail" in ablate:
                    bstate.pop(s, None)
                    return
                accs, lse_sb = bstate.pop(s)
                # num1 = sum(ww * lse): single fused STT with accumulate
                t_sb = t_pool.tile([128, MB * L], bf16, tag="t")
                nc.vector.scalar_tensor_tensor(
                    out=t_sb,
                    in0=ww_sb.rearrange("p m l -> p (m l)"),
                    scalar=1.0,
                    in1=lse_sb.rearrange("p m l -> p (m l)"),
                    op0=MULT, op1=MULT,
                    accum_out=res_sb[:, bt, 0:1],
                )
                if "select" in ablate:
                    pass
                elif fast_cw:
                    nc.vector.tensor_reduce(
                        out=res_sb[:, bt, 1:2], in_=accs[0],
                        axis=mybir.AxisListType.X, op=ADD,
                    )
                else:
                    nc.vector.tensor_reduce(
                        out=res_sb[:, bt, 1:7],
                        in_=accs[0].rearrange("p (m c) -> p c m", c=6),
                        axis=mybir.AxisListType.X, op=ADD,
                    )

            # Software pipeline, 2 stages deep. Per-engine instruction order
            # interleaves stage s-1 consumption with stage s projection at mb
            # granularity so no engine's in-order stream blocks another:
            #   PE : [logits(s-1,mb)] [proj(s,dbs)] x4
            #   ACT: [exp(s-1)] [qk copies(s)] x4, then Ln(s-1)
            #   DVE: [num1/reduce(s-2)] [selects(s-1), tree L1] x4
            #   Pool: tree L2/L3(s-1)
            DBS = ((0, 1), (2, 3), (4,), (5,))
            for s in range(S + 1):
                if s < S:
                    front(s)
                for mb in range(MB):
                    if s >= 1:
                        back_mb(s - 1, mb)
                        if mb == 0 and s >= 2:
                            tail_dve(s - 2)
                    if s < S:
                        proj_part(s, DBS[mb])
                if s >= 1:
                    tail_ln(s - 1)
            tail_dve(S - 1)

            # cross-partition reduce via ones-matmul (fp32, tiny)
            pout = pproj_pool.tile([1, nres * BT_PER_CORE], f32, tag="proj")
            nc.tensor.matmul(
                pout,
                lhsT=ones_sb[:, :],
                rhs=res_sb.rearrange("p b k -> p (b k)"),
                start=True, stop=True,
            )
            out_sb = res_pool.tile([1, nres * BT_PER_CORE], f32)
            nc.vector.tensor_copy(out_sb, pout)
            nc.sync.dma_start(out=out_d[:, :], in_=out_sb)

    nc.compile()
    nc.finalize()
    return nc


def _prep_core_inputs(x, W, b, class_weight, labels, mask):
    """Host-side prep. Returns (in_maps list of 8 dicts, den[32] float64)."""
    x32 = np.ascontiguousarray(np.asarray(x, np.float32).reshape(B * T, L, H))
    labels32 = np.asarray(labels).reshape(B * T, L, L)
    mask32 = np.asarray(mask).reshape(B * T, L, L)

    cw = np.asarray(class_weight, np.float32)
    fast_cw = bool(np.all(cw[1:] == cw[1]))

    Wr = np.asarray(W, np.float32).reshape(H, C, 4, INNER)
    Wq = Wr[:, :, 0, :].reshape(H, C, INNER).copy()
    br = np.asarray(b, np.float32).reshape(C, 4, INNER)
    bq = br[:, 0, :].copy()
    if fast_cw:
        # pre-scale class-0 q so the select accumulates cw[lab]/w * logit;
        # the device exp undoes this with scale=w/a (exact for a/w = 0.25)
        Wq[:, 0, :] *= cw[0] / cw[1]
        bq[0, :] *= cw[0] / cw[1]
    Wq = Wq.reshape(H, C * INNER)
    Wk = Wr[:, :, 2, :].reshape(H, C * INNER)
    if _FP8PROJ:
        wqk = np.ascontiguousarray(
            (np.concatenate([Wq, Wk], axis=1) * 64.0).reshape(3, 2, 128, 768)
        ).astype(_F8)
    else:
        wqk = np.ascontiguousarray(
            np.concatenate([Wq, Wk], axis=1).reshape(HC, 128, 768)
        ).astype(_BF16)

    br = np.concatenate([bq.ravel(), br[:, 2, :].ravel()])
    bias = np.ascontiguousarray(br.reshape(6, 128).T).astype(np.float32)
    ww_all = (cw[labels32] * mask32).astype(np.float32)          # [32, L, L]
    den = ww_all.astype(np.float64).reshape(B * T, -1).sum(axis=1)

    # masked labels: mask==0 positions get +32 so they never match any class;
    # lab3[..., j, :] = lab' - j for the 2-plane chunked select
    labp = (labels32 + 32 * (1 - mask32)).astype(np.float32)
    j2 = np.arange(2, dtype=np.float32).reshape(1, 1, 2, 1)

    in_maps = []
    for core in range(NCORES):
        sl = slice(core * BT_PER_CORE, (core + 1) * BT_PER_CORE)
        if _FP8PROJ:
            xt = np.ascontiguousarray(
                x32[sl].transpose(0, 2, 1).reshape(BT_PER_CORE, 3, 2, 128, L)
            ).astype(_F8)
        else:
            xt = np.ascontiguousarray(
                x32[sl].transpose(0, 2, 1).reshape(BT_PER_CORE, HC, 128, L)
            ).astype(_BF16)
        lab3 = np.ascontiguousarray(
            labp[sl].reshape(BT_PER_CORE, MB, 128, 1, L) - j2[None]
        ).astype(_BF16)
        ww_s = np.ascontiguousarray(
            ww_all[sl].reshape(BT_PER_CORE, MB, 128, L)
        ).astype(_BF16)
        in_maps.append({"wqk": wqk, "bias": bias, "xt": xt, "lab3": lab3, "ww": ww_s})
    return in_maps, den


def kernel(x, W, b, class_weight, labels, mask):
    from concourse.bass_utils import run_bass_kernel_spmd

    cw = np.asarray(class_weight, np.float64)
    fast_cw = bool(np.all(cw[1:] == cw[1]))
    b_zero = bool(np.all(np.asarray(b) == 0.0))
    w_over_a = float(cw[1] / cw[0]) if fast_cw else 4.0
    key = ("nc", fast_cw, b_zero, w_over_a)
    if key not in _compiled:
        _compiled[key] = _build_nc(fast_cw, b_zero=b_zero, w_over_a=w_over_a)
    nc = _compiled[key]

    in_maps, den = _prep_core_inputs(x, W, b, class_weight, labels, mask)
    res = run_bass_kernel_spmd(nc, in_maps, core_ids=list(range(NCORES)))
    _compiled["last_res"] = res

    nres = 2 if fast_cw else 7
    loss = 0.0
    for core in range(NCORES):
        out = np.asarray(res.results[core]["out"], np.float64).reshape(BT_PER_CORE, nres)
        for i in range(BT_PER_CORE):
            num1 = out[i, 0]
            if fast_cw:
                num2 = cw[1] * out[i, 1]
            else:
                num2 = float(cw @ out[i, 1:7])
            d = max(den[core * BT_PER_CORE + i], 1e-9)
            loss += (num1 - num2) / d
    return np.float32(loss)
